# revision 11
# baseline (speedup 1.0000x reference)
"""LiquidEmbedding Trainium2 kernel.

Pipeline per sample (B=16, S=2048, E=512, vocab=50257):
  x = tok_emb[tokens] + pos_emb                      (indirect-DMA gather)
  h = relu(conv1d(relu(conv1d(x))))                  (3-tap convs as matmuls)
  X = FFT_S(h)  -> mag stats -> per-sample cutoff N  (DFT as matmul, bf16)
  out = ifft(X * prefix_mask(N)).real @ proj_w.T     (masked inverse DFT + proj)
  loss_recon = 0 exactly (reference compares two identical computations)

Key math facts used:
  * ratio = clip(0.5*(1-complexity), 0.1, 1.0) <= 0.5  ->  N <= 1024, so only
    FFT bins 0..1023 can survive the mask; bin 1024 and the conjugate mirror
    bins 1025..2047 are only needed for the magnitude statistics, where
    mag[S-k] = mag[k] (real input) lets us count k=1..1023 twice.
  * The masked-inverse-FFT real part is  (1/S)(Ci_cos^T (m*R) + Ci_sin^T (m*I))
    with R = sum_n cos(2pi nk/S) h[n], I = sum_n sin(2pi nk/S) h[n].
  * N = floor(ratio*S) and the prefix mask k < N is equivalent to the float
    compare (k+1) <= ratio*S, so N is never materialized as an integer.

Sharding: pure data parallel, 2 samples per core on 8 cores.
"""

import os
import sys

import numpy as np

for _p in ("/opt/trn_rl_repo", "/root/.axon_site/_ro/trn_rl_repo", "/root/.axon_site"):
    if os.path.isdir(_p) and _p not in sys.path:
        sys.path.insert(0, _p)

import ml_dtypes  # noqa: E402
from contextlib import ExitStack  # noqa: E402

import concourse.bass as bass  # noqa: E402
import concourse.tile as tile  # noqa: E402
from concourse import bacc, mybir  # noqa: E402
from concourse.bass_utils import run_bass_kernel_spmd  # noqa: E402
from concourse.masks import make_identity  # noqa: E402

V, E, S = 50257, 512, 2048
B = 16
NCORES = 8
BPC = B // NCORES          # samples per core
NT = S // 128              # 16 seq tiles
ET = E // 128              # 4 embed tiles
KT = 1024 // 128           # 8 freq tiles (bins 0..1023)
F32 = mybir.dt.float32
BF16 = mybir.dt.bfloat16
I32 = mybir.dt.int32
AF = mybir.ActivationFunctionType
ALU = mybir.AluOpType

_CACHE = {}


def _build_kernel(ctx: ExitStack, tc: tile.TileContext, outs, ins):
    nc = tc.nc
    out_d = outs["out"]          # [BPC*S, E] f32
    loss_d = outs["loss"]        # [1, 1] f32
    tok_emb = ins["tok_emb"]     # [V, E] f32
    tokens = ins["tokens"]       # [128, BPC*NT] i32 (col b*NT+t, part p -> token[t*128+p])
    pos = ins["pos"]             # [S, E] f32
    w1t = ins["w1t"]             # [3, E, E] bf16 (tap, ei, eo)
    w2t = ins["w2t"]             # [3, E, E] bf16
    pj = ins["pj"]               # [E, E] bf16 (e, eo) = proj_w.T
    cfc = ins["cfc"]             # [S, 1024] bf16 cos(2pi n k/S)
    cfs = ins["cfs"]             # [S, 1024] bf16 sin(2pi n k/S)
    c1024 = ins["c1024"]         # [S, 1] bf16 (-1)^n
    cic = ins["cic"]             # [1024, S] bf16 cos(2pi k n/S)/S
    cis = ins["cis"]             # [1024, S] bf16 sin(2pi k n/S)/S

    const = ctx.enter_context(tc.tile_pool(name="const", bufs=1))
    work = ctx.enter_context(tc.tile_pool(name="work", bufs=1))
    gath = ctx.enter_context(tc.tile_pool(name="gath", bufs=6))
    strm = ctx.enter_context(tc.tile_pool(name="strm", bufs=3))
    outp = ctx.enter_context(tc.tile_pool(name="outp", bufs=3))
    stat = ctx.enter_context(tc.tile_pool(name="stat", bufs=1))
    psum = ctx.enter_context(tc.tile_pool(name="psum", bufs=8, space="PSUM"))

    # ---- constants / weights (loaded once) ----
    id128 = const.tile([128, 128], BF16)
    make_identity(nc, id128)
    km1f = const.tile([128, 1024], F32)
    km1i = strm.tile([128, 1024], I32, tag="cib")
    nc.gpsimd.iota(km1i[:, :], pattern=[[1, 1024]], base=1, channel_multiplier=0)
    nc.vector.tensor_copy(km1f, km1i[:, :])
    ones_col = const.tile([128, 1], F32)
    nc.vector.memset(ones_col, 1.0)
    ones_row = const.tile([1, 128], F32)
    nc.vector.memset(ones_row, 1.0)
    z11 = const.tile([1, 1], F32)
    nc.vector.memset(z11, 0.0)
    nc.sync.dma_start(loss_d[:, :], z11[:, :])

    tok_sb = const.tile([128, BPC * NT], I32)
    nc.sync.dma_start(tok_sb[:, :], tokens[:, :])
    w1_sb = const.tile([128, 3 * ET * E], BF16)
    nc.sync.dma_start(
        w1_sb.rearrange("p (t a o) -> p t a o", t=3, a=ET),
        w1t.rearrange("t (a p) o -> p t a o", a=ET),
    )
    w2_sb = const.tile([128, 3 * ET * E], BF16)
    nc.sync.dma_start(
        w2_sb.rearrange("p (t a o) -> p t a o", t=3, a=ET),
        w2t.rearrange("t (a p) o -> p t a o", a=ET),
    )
    pj_sb = const.tile([128, ET * E], BF16)
    nc.sync.dma_start(
        pj_sb.rearrange("p (a o) -> p a o", a=ET),
        pj.rearrange("(a p) o -> p a o", a=ET),
    )
    pos_sb = const.tile([128, NT * E], BF16)
    nc.sync.dma_start(
        pos_sb.rearrange("p (t e) -> p t e", t=NT),
        pos.rearrange("(t p) e -> p t e", t=NT),
    )
    c1024_sb = const.tile([128, NT], BF16)
    nc.sync.dma_start(
        c1024_sb.rearrange("p (t o) -> p t o", t=NT),
        c1024.rearrange("(t p) o -> p t o", t=NT),
    )

    CW = 2052  # padded conv row stride (cols 0 and 2049.. are zero)

    for b in range(BPC):
        # ---- stage A: gather + pos add + transpose to [e, s] ----
        xTp = work.tile([128, ET * CW], BF16, tag="xTp")
        for j in range(ET):
            nc.vector.memset(xTp[:, j * CW : j * CW + 1], 0.0)
            nc.vector.memset(xTp[:, j * CW + 2049 : (j + 1) * CW], 0.0)
        for t in range(NT):
            xg = gath.tile([128, E], F32, tag="xg")
            nc.gpsimd.indirect_dma_start(
                out=xg[:, :],
                out_offset=None,
                in_=tok_emb[:, :],
                in_offset=bass.IndirectOffsetOnAxis(
                    ap=tok_sb[:, b * NT + t : b * NT + t + 1], axis=0
                ),
            )
            xb = gath.tile([128, E], BF16, tag="xb")
            nc.vector.tensor_tensor(
                xb[:, :], xg[:, :], pos_sb[:, t * E : (t + 1) * E], op=ALU.add
            )
            for j in range(ET):
                pt = psum.tile([128, 512], BF16, tag="ps")
                nc.tensor.transpose(
                    pt[:, :128], xb[:, j * 128 : (j + 1) * 128], id128[:, :]
                )
                nc.any.tensor_copy(
                    xTp[:, j * CW + 1 + t * 128 : j * CW + 1 + (t + 1) * 128],
                    pt[:, :128],
                )

        # ---- stage B: conv1 -> h1p [eo, s] (padded) ----
        h1p = work.tile([128, ET * CW], BF16, tag="h1p")
        for j in range(ET):
            nc.vector.memset(h1p[:, j * CW : j * CW + 1], 0.0)
            nc.vector.memset(h1p[:, j * CW + 2049 : (j + 1) * CW], 0.0)
        for eo in range(ET):
            for sc in range(4):
                ps = psum.tile([128, 512], F32, tag="ps")
                k = 0
                for tap in range(3):
                    for ei in range(ET):
                        nc.tensor.matmul(
                            ps[:, :],
                            lhsT=w1_sb[
                                :,
                                (tap * ET + ei) * E + eo * 128 : (tap * ET + ei) * E
                                + (eo + 1) * 128,
                            ],
                            rhs=xTp[
                                :, ei * CW + sc * 512 + tap : ei * CW + sc * 512 + tap + 512
                            ],
                            start=(k == 0),
                            stop=(k == 11),
                        )
                        k += 1
                nc.scalar.activation(
                    h1p[:, eo * CW + 1 + sc * 512 : eo * CW + 1 + (sc + 1) * 512],
                    ps[:, :],
                    AF.Relu,
                )

        # ---- stage C: conv2 -> H [s, e] ----
        Hsb = work.tile([128, NT * E], BF16, tag="Hsb")
        for st in range(NT):
            ps = psum.tile([128, 512], F32, tag="ps")
            k = 0
            for tap in range(3):
                for ei in range(ET):
                    nc.tensor.matmul(
                        ps[:, :],
                        lhsT=h1p[
                            :, ei * CW + st * 128 + tap : ei * CW + (st + 1) * 128 + tap
                        ],
                        rhs=w2_sb[:, (tap * ET + ei) * E : (tap * ET + ei + 1) * E],
                        start=(k == 0),
                        stop=(k == 11),
                    )
                    k += 1
            nc.scalar.activation(Hsb[:, st * E : (st + 1) * E], ps[:, :], AF.Relu)

        # ---- stage D: bin 1024 (X[1024] = sum (-1)^n h[n], imag = 0) ----
        ps4 = psum.tile([128, 512], F32, tag="ps")
        for et in range(ET):
            for nt in range(NT):
                nc.tensor.matmul(
                    ps4[:, et : et + 1],
                    lhsT=Hsb[:, nt * E + et * 128 : nt * E + (et + 1) * 128],
                    rhs=c1024_sb[:, nt : nt + 1],
                    start=(nt == 0),
                    stop=(nt == NT - 1),
                )
        x2_1024 = stat.tile([128, ET], F32, tag="x2_1024")
        nc.scalar.activation(x2_1024[:, :], ps4[:, :ET], AF.Square)

        # ---- stage E: forward DFT -> RT/IT [e, k] bf16 (k = 0..1023) ----
        RT = work.tile([128, ET * 1024], BF16, tag="RT")
        IT = work.tile([128, ET * 1024], BF16, tag="IT")
        for kc in range(2):
            pcs = []
            for et in range(ET):
                pc = psum.tile([128, 512], F32, tag="ps", name=f"pc{et}")
                psn = psum.tile([128, 512], F32, tag="ps", name=f"psn{et}")
                pcs.append((pc, psn))
            for nt in range(NT):
                cfb = strm.tile([128, 1024], BF16, tag="cfb")
                nc.sync.dma_start(
                    cfb[:, 0:512],
                    cfc[nt * 128 : (nt + 1) * 128, kc * 512 : (kc + 1) * 512],
                )
                nc.sync.dma_start(
                    cfb[:, 512:1024],
                    cfs[nt * 128 : (nt + 1) * 128, kc * 512 : (kc + 1) * 512],
                )
                for et in range(ET):
                    lhsT = Hsb[:, nt * E + et * 128 : nt * E + (et + 1) * 128]
                    nc.tensor.matmul(
                        pcs[et][0][:, :], lhsT=lhsT, rhs=cfb[:, 0:512],
                        start=(nt == 0), stop=(nt == NT - 1),
                    )
                    nc.tensor.matmul(
                        pcs[et][1][:, :], lhsT=lhsT, rhs=cfb[:, 512:1024],
                        start=(nt == 0), stop=(nt == NT - 1),
                    )
            for et in range(ET):
                nc.any.tensor_copy(
                    RT[:, et * 1024 + kc * 512 : et * 1024 + (kc + 1) * 512],
                    pcs[et][0][:, :],
                )
                nc.any.tensor_copy(
                    IT[:, et * 1024 + kc * 512 : et * 1024 + (kc + 1) * 512],
                    pcs[et][1][:, :],
                )

        # ---- stage F: magnitude stats -> scalar x = ratio*S, mask ----
        # per-e-block: mag2 -> max -> thresh -> counts (block scratch reused)
        mx = stat.tile([128, ET], F32, tag="mx")
        th2 = stat.tile([128, ET], F32, tag="th2")
        cntA = stat.tile([128, ET], F32, tag="cntA")
        cnt0 = stat.tile([128, ET], F32, tag="cnt0")
        for j in range(ET):
            blk = slice(j * 1024, (j + 1) * 1024)
            m2j = stat.tile([128, 1024], BF16, tag="m2j")
            sq = stat.tile([128, 1024], BF16, tag="sq")
            nc.vector.tensor_tensor(m2j[:, :], RT[:, blk], RT[:, blk], op=ALU.mult)
            nc.vector.tensor_tensor(sq[:, :], IT[:, blk], IT[:, blk], op=ALU.mult)
            nc.vector.tensor_tensor(m2j[:, :], m2j[:, :], sq[:, :], op=ALU.add)
            nc.vector.tensor_reduce(
                mx[:, j : j + 1], m2j[:, :], axis=mybir.AxisListType.X, op=ALU.max
            )
            nc.vector.tensor_tensor(
                mx[:, j : j + 1], mx[:, j : j + 1], x2_1024[:, j : j + 1], op=ALU.max
            )
            nc.vector.tensor_scalar_mul(th2[:, j : j + 1], mx[:, j : j + 1], 0.01)
            c01 = stat.tile([128, 1024], BF16, tag="c01")
            nc.vector.tensor_scalar(
                c01[:, :], m2j[:, :], th2[:, j : j + 1], None, op0=ALU.is_gt,
                op1=ALU.add, accum_out=cntA[:, j : j + 1],
            )
            nc.vector.tensor_scalar(
                cnt0[:, j : j + 1],
                m2j[:, 0:1],
                th2[:, j : j + 1],
                None,
                op0=ALU.is_gt,
            )
        cnt1024 = stat.tile([128, ET], F32, tag="cnt1024")
        nc.vector.tensor_tensor(cnt1024[:, :], x2_1024[:, :], th2[:, :], op=ALU.is_gt)
        wsum = stat.tile([128, ET], F32, tag="wsum")
        nc.vector.tensor_scalar_mul(wsum[:, :], cntA[:, :], 2.0)
        nc.vector.tensor_tensor(wsum[:, :], wsum[:, :], cnt0[:, :], op=ALU.subtract)
        nc.vector.tensor_tensor(wsum[:, :], wsum[:, :], cnt1024[:, :], op=ALU.add)
        totp = stat.tile([128, 1], F32, tag="totp")
        nc.vector.tensor_reduce(
            totp[:, :], wsum[:, :], axis=mybir.AxisListType.X, op=ALU.add
        )
        pt11 = psum.tile([128, 512], F32, tag="ps")
        nc.tensor.matmul(
            pt11[:1, :1], lhsT=ones_col[:, :], rhs=totp[:, :], start=True, stop=True
        )
        # scalar chain: c = tot/2^20 ; r = max(0.1, c*-0.5+0.5) ; x = r*2048
        sc1 = stat.tile([1, 1], F32, tag="sc1")
        nc.scalar.activation(sc1[:, :], pt11[:1, :1], AF.Copy, scale=1.0 / 1048576.0)
        sc2 = stat.tile([1, 1], F32, tag="sc2")
        nc.scalar.activation(sc2[:, :], sc1[:, :], AF.Copy, scale=-0.5, bias=0.5)
        nc.vector.tensor_scalar_max(sc2[:, :], sc2[:, :], 0.1)
        nc.scalar.activation(sc2[:, :], sc2[:, :], AF.Copy, scale=2048.0)
        pbx = psum.tile([128, 512], F32, tag="ps")
        nc.tensor.matmul(
            pbx[:, :1], lhsT=ones_row[:, :], rhs=sc2[:, :], start=True, stop=True
        )
        xb128 = stat.tile([128, 1], F32, tag="xb128")
        nc.vector.tensor_copy(xb128[:, :], pbx[:, :1])
        maskf = stat.tile([128, 1024], BF16, tag="maskf")
        nc.vector.tensor_scalar(
            maskf[:, :], km1f[:, :], xb128[:, :], None, op0=ALU.is_le
        )
        for j in range(ET):
            blk = slice(j * 1024, (j + 1) * 1024)
            nc.vector.tensor_tensor(RT[:, blk], RT[:, blk], maskf[:, :], op=ALU.mult)
            nc.vector.tensor_tensor(IT[:, blk], IT[:, blk], maskf[:, :], op=ALU.mult)

        # ---- stage G: freq-domain projection -> Rpp/Ipp [k, eo] bf16 ----
        Rpp = work.tile([128, KT * E], BF16, tag="Rpp")
        Ipp = work.tile([128, KT * E], BF16, tag="Ipp")
        for src, dst in ((RT, Rpp), (IT, Ipp)):
            for kt in range(KT):
                ps = psum.tile([128, 512], F32, tag="ps")
                for et in range(ET):
                    nc.tensor.matmul(
                        ps[:, :],
                        lhsT=src[:, et * 1024 + kt * 128 : et * 1024 + (kt + 1) * 128],
                        rhs=pj_sb[:, et * E : (et + 1) * E],
                        start=(et == 0),
                        stop=(et == ET - 1),
                    )
                nc.any.tensor_copy(dst[:, kt * E : (kt + 1) * E], ps[:, :])

        # ---- stage H: masked inverse DFT -> out [n, eo] f32 -> DRAM ----
        for nt in range(NT):
            cib = strm.tile([128, 2048], BF16, tag="cib")
            nc.sync.dma_start(
                cib[:, 0:1024].rearrange("p (a n) -> p a n", a=KT),
                cic.rearrange("(a p) n -> p a n", a=KT)[
                    :, :, nt * 128 : (nt + 1) * 128
                ],
            )
            nc.sync.dma_start(
                cib[:, 1024:2048].rearrange("p (a n) -> p a n", a=KT),
                cis.rearrange("(a p) n -> p a n", a=KT)[
                    :, :, nt * 128 : (nt + 1) * 128
                ],
            )
            ps = psum.tile([128, 512], F32, tag="ps")
            for kt in range(KT):
                nc.tensor.matmul(
                    ps[:, :],
                    lhsT=cib[:, kt * 128 : (kt + 1) * 128],
                    rhs=Rpp[:, kt * E : (kt + 1) * E],
                    start=(kt == 0),
                    stop=False,
                )
                nc.tensor.matmul(
                    ps[:, :],
                    lhsT=cib[:, 1024 + kt * 128 : 1024 + (kt + 1) * 128],
                    rhs=Ipp[:, kt * E : (kt + 1) * E],
                    start=False,
                    stop=(kt == KT - 1),
                )
            ob = outp.tile([128, E], F32, tag="ob")
            nc.any.tensor_copy(ob[:, :], ps[:, :])
            nc.sync.dma_start(
                out_d[b * S + nt * 128 : b * S + (nt + 1) * 128, :], ob[:, :]
            )


def _get_compiled():
    if "nc" in _CACHE:
        return _CACHE["nc"]
    nc = bacc.Bacc(
        "TRN2", target_bir_lowering=False, debug=False, enable_asserts=False
    )
    ins = {
        "tok_emb": nc.dram_tensor("tok_emb", [V, E], F32, kind="ExternalInput").ap(),
        "tokens": nc.dram_tensor("tokens", [128, BPC * NT], I32, kind="ExternalInput").ap(),
        "pos": nc.dram_tensor("pos", [S, E], BF16, kind="ExternalInput").ap(),
        "w1t": nc.dram_tensor("w1t", [3, E, E], BF16, kind="ExternalInput").ap(),
        "w2t": nc.dram_tensor("w2t", [3, E, E], BF16, kind="ExternalInput").ap(),
        "pj": nc.dram_tensor("pj", [E, E], BF16, kind="ExternalInput").ap(),
        "cfc": nc.dram_tensor("cfc", [S, 1024], BF16, kind="ExternalInput").ap(),
        "cfs": nc.dram_tensor("cfs", [S, 1024], BF16, kind="ExternalInput").ap(),
        "c1024": nc.dram_tensor("c1024", [S, 1], BF16, kind="ExternalInput").ap(),
        "cic": nc.dram_tensor("cic", [1024, S], BF16, kind="ExternalInput").ap(),
        "cis": nc.dram_tensor("cis", [1024, S], BF16, kind="ExternalInput").ap(),
    }
    outs = {
        "out": nc.dram_tensor("out", [BPC * S, E], F32, kind="ExternalOutput").ap(),
        "loss": nc.dram_tensor("loss", [1, 1], F32, kind="ExternalOutput").ap(),
    }
    with tile.TileContext(nc) as tc:
        with ExitStack() as ctx:
            _build_kernel(ctx, tc, outs, ins)
    nc.compile()
    _CACHE["nc"] = nc
    return nc


def _host_prep(inputs):
    """Build the per-core in_maps (host-side constant prep + sharding)."""
    tokens = np.asarray(inputs["tokens"]).astype(np.int32)
    tok_emb = np.ascontiguousarray(np.asarray(inputs["tok_emb"], dtype=np.float32))
    pos_emb = np.ascontiguousarray(np.asarray(inputs["pos_emb"], dtype=np.float32))
    pos_bf = pos_emb.astype(ml_dtypes.bfloat16)
    bf = ml_dtypes.bfloat16
    w1t = np.ascontiguousarray(
        np.asarray(inputs["conv1_w"], dtype=np.float32).transpose(2, 1, 0)
    ).astype(bf)  # [tap, ei, eo]
    w2t = np.ascontiguousarray(
        np.asarray(inputs["conv2_w"], dtype=np.float32).transpose(2, 1, 0)
    ).astype(bf)
    pj = np.ascontiguousarray(
        np.asarray(inputs["proj_w"], dtype=np.float32).T
    ).astype(bf)  # [e, eo]

    n = np.arange(S, dtype=np.float64)
    k = np.arange(1024, dtype=np.float64)
    ang = 2.0 * np.pi / S * np.outer(n, k)  # [S, 1024]
    cfc = np.cos(ang).astype(bf)
    cfs = np.sin(ang).astype(bf)
    c1024 = np.where(n.astype(np.int64) % 2 == 0, 1.0, -1.0).reshape(S, 1).astype(bf)
    angi = 2.0 * np.pi / S * np.outer(k, n)  # [1024, S]
    cic = (np.cos(angi) / S).astype(bf)
    cis = (np.sin(angi) / S).astype(bf)

    shared = {
        "tok_emb": tok_emb, "pos": pos_bf, "w1t": w1t, "w2t": w2t, "pj": pj,
        "cfc": cfc, "cfs": cfs, "c1024": c1024, "cic": cic, "cis": cis,
    }
    in_maps = []
    for c in range(NCORES):
        tc_ = tokens[c * BPC : (c + 1) * BPC]  # [BPC, S]
        tok_packed = np.ascontiguousarray(
            tc_.reshape(BPC, NT, 128).transpose(2, 0, 1).reshape(128, BPC * NT)
        )
        in_maps.append(dict(shared, tokens=tok_packed))
    return in_maps


def kernel(**inputs):
    nc = _get_compiled()
    in_maps = _host_prep(inputs)
    res = run_bass_kernel_spmd(nc, in_maps, core_ids=list(range(NCORES)))
    out = np.concatenate(
        [r["out"].reshape(BPC, S, E) for r in res.results], axis=0
    ).astype(np.float32)
    loss = np.float32(res.results[0]["loss"].reshape(()))
    return out, loss
